# revision 5
# baseline (speedup 1.0000x reference)
"""Trainium2 Bass kernel for nn_ContrastiveLoss (N=8192, D=1024, 751 ids).

loss = (1/N) * sum_ij [ same(i,j) & sim<1 -> (1-sim) ; diff(i,j) & sim>0.3 -> sim ]
with sim = X @ X.T.

Strategy (8 NeuronCores, data-parallel rows):
  * Host: sort rows by label (loss is permutation invariant). Same-label
    pairs then live within +-63 of the diagonal (max class count ~28).
  * Each core computes its [1024 x 8192] slab of sim via fp16 matmul
    (fp32 PSUM accumulate).
  * Unmasked full-slab term: sum_j sim*1[sim>0.3]
      = sum relu(sim-0.3) + 0.3 * count(sim>0.3),
    computed with two ScalarE activations (Relu / Sign, bias=-0.3) with
    fused free-dim accumulation (accum_out). No mask needed.
  * Band correction (256-wide windows around the diagonal): for
    same-label pairs subtract the neg term and add relu(1-sim), using a
    device-side label-equality mask (fp16 labels, is_equal).
  * Host: gather [128,8] fp32 partials per core, reduce in float64.
"""

import sys

for _p in ("/opt/trn_rl_repo",):
    if _p not in sys.path:
        sys.path.append(_p)

import numpy as np

import concourse.bass as bass
import concourse.mybir as mybir
import concourse.tile as tile
from concourse import bacc
from concourse.bass_utils import run_bass_kernel_spmd

N = 8192          # rows
D = 1024          # feature dim
NCORES = 8
RPC = N // NCORES  # rows per core = 1024
MT = RPC // 128    # m-tiles per core = 8
KT = D // 128      # contraction chunks = 8
NG = 4             # rhs column groups
GW = N // NG       # group width = 2048
NPG = GW // 512    # 512-wide n-tiles per group = 4
BW = 256           # band window width
MARGIN = 0.3

f16 = mybir.dt.float16
f32 = mybir.dt.float32

_CACHE = {}


def _build_program():
    nc = bacc.Bacc("TRN2", target_bir_lowering=False, debug=False,
                   num_devices=NCORES)

    xt = nc.dram_tensor("xt", [D, N], f16, kind="ExternalInput")
    lhs = nc.dram_tensor("lhs", [D, RPC], f16, kind="ExternalInput")
    bwin = nc.dram_tensor("bwin", [D, MT * BW], f16, kind="ExternalInput")
    wlab = nc.dram_tensor("wlab", [MT * 128, BW], f16, kind="ExternalInput")
    rlab = nc.dram_tensor("rlab", [128, MT], f32, kind="ExternalInput")
    outp = nc.dram_tensor("out", [128, 8], f32, kind="ExternalOutput")

    xt_t = xt.rearrange("(k p) n -> k p n", p=128)
    lhs_t = lhs.rearrange("(k p) m -> k p m", p=128)
    bwin_t = bwin.rearrange("(k p) w -> k p w", p=128)
    wlab_t = wlab.rearrange("(j p) w -> j p w", p=128)

    Relu = mybir.ActivationFunctionType.Relu
    Sign = mybir.ActivationFunctionType.Sign
    Op = mybir.AluOpType

    with tile.TileContext(nc) as tc:
        with (
            tc.tile_pool(name="persist", bufs=1) as persist,
            tc.tile_pool(name="rhsp", bufs=2) as rhsp,
            tc.tile_pool(name="scr", bufs=8) as scr,
            tc.tile_pool(name="band", bufs=3) as bandp,
            tc.tile_pool(name="psum_m", bufs=6, space="PSUM") as psum_m,
            tc.tile_pool(name="psum_b", bufs=2, space="PSUM") as psum_b,
        ):
            # ---- persistent loads ----
            lhs_sb = []
            for k in range(KT):
                tl = persist.tile([128, RPC], f16, name=f"lhs{k}")
                nc.sync.dma_start(tl[:], lhs_t[k])
                lhs_sb.append(tl)
            bwin_sb = []
            for k in range(KT):
                tb = persist.tile([128, MT * BW], f16, name=f"bwin{k}")
                nc.sync.dma_start(tb[:], bwin_t[k])
                bwin_sb.append(tb)
            wlab_sb = []
            for j in range(MT):
                tw = persist.tile([128, BW], f16, name=f"wlab{j}")
                nc.sync.dma_start(tw[:], wlab_t[j])
                wlab_sb.append(tw)
            rlab_sb = persist.tile([128, MT], f32, name="rlab")
            nc.sync.dma_start(rlab_sb[:], rlab[:])

            stats_r = persist.tile([128, NG * MT * NPG], f32, name="stats_r")
            stats_s = persist.tile([128, NG * MT * NPG], f32, name="stats_s")
            stats_b = persist.tile([128, MT], f32, name="stats_b")
            out_t = persist.tile([128, 8], f32, name="out_t")
            bias_m = persist.tile([128, 1], f32, name="bias_m")
            nc.vector.memset(bias_m[:], -MARGIN)

            # ---- band: sim on [128 x 256] diagonal windows ----
            for j in range(MT):
                ps = psum_b.tile([128, BW], f32)
                for k in range(KT):
                    nc.tensor.matmul(
                        ps[:],
                        lhs_sb[k][:, j * 128:(j + 1) * 128],
                        bwin_sb[k][:, j * BW:(j + 1) * BW],
                        start=(k == 0), stop=(k == KT - 1),
                    )
                pos = bandp.tile([128, BW], f32, name="pos")
                rb = bandp.tile([128, BW], f32, name="rb")
                sg = bandp.tile([128, BW], f32, name="sg")
                # pos = relu(1 - s);  rb = relu(s - 0.3);  sg = sign(s - 0.3)
                nc.scalar.activation(pos[:], ps[:], Relu, bias=1.0, scale=-1.0)
                nc.scalar.activation(rb[:], ps[:], Relu, bias=bias_m[:])
                nc.scalar.activation(sg[:], ps[:], Sign, bias=bias_m[:])
                # neg = rb + 0.15*sg + 0.15 ; corr = eq * (pos - neg)
                a = bandp.tile([128, BW], f32, name="a")
                nc.vector.scalar_tensor_tensor(
                    a[:], sg[:], 0.15, pos[:], op0=Op.mult, op1=Op.subtract)
                b = bandp.tile([128, BW], f32, name="b")
                nc.vector.scalar_tensor_tensor(
                    b[:], a[:], 0.15, rb[:], op0=Op.add, op1=Op.add)
                # b = neg - pos
                eq = bandp.tile([128, BW], f32, name="eq")
                nc.vector.tensor_scalar(
                    eq[:], wlab_sb[j][:], rlab_sb[:, j:j + 1], None,
                    op0=Op.is_equal)
                crr = bandp.tile([128, BW], f32, name="crr")
                nc.vector.scalar_tensor_tensor(
                    crr[:], b[:], -1.0, eq[:], op0=Op.mult, op1=Op.mult,
                    accum_out=stats_b[:, j:j + 1])

            # ---- main sweep: full [1024 x 8192] slab ----
            for g in range(NG):
                rhs_g = []
                for k in range(KT):
                    tr = rhsp.tile([128, GW], f16, name=f"rhs{k}")
                    nc.sync.dma_start(tr[:], xt_t[k, :, g * GW:(g + 1) * GW])
                    rhs_g.append(tr)
                for m in range(MT):
                    ps4 = [psum_m.tile([128, 512], f32, name="mm")
                           for i in range(NPG)]
                    for k in range(KT):
                        for i in range(NPG):
                            nc.tensor.matmul(
                                ps4[i][:],
                                lhs_sb[k][:, m * 128:(m + 1) * 128],
                                rhs_g[k][:, i * 512:(i + 1) * 512],
                                start=(k == 0), stop=(k == KT - 1),
                            )
                    for i in range(NPG):
                        col = (g * MT + m) * NPG + i
                        sr = scr.tile([128, 512], f16, name="sr")
                        nc.scalar.activation(
                            sr[:], ps4[i][:], Relu, bias=bias_m[:],
                            accum_out=stats_r[:, col:col + 1])
                        ss = scr.tile([128, 512], f16, name="ss")
                        nc.scalar.activation(
                            ss[:], ps4[i][:], Sign, bias=bias_m[:],
                            accum_out=stats_s[:, col:col + 1])

            # ---- final per-core reduction -> [128, 8] ----
            nc.vector.memset(out_t[:], 0.0)
            nc.vector.tensor_reduce(
                out_t[:, 0:1], stats_r[:], axis=mybir.AxisListType.X,
                op=Op.add)
            nc.vector.tensor_reduce(
                out_t[:, 1:2], stats_s[:], axis=mybir.AxisListType.X,
                op=Op.add)
            nc.vector.tensor_reduce(
                out_t[:, 2:3], stats_b[:], axis=mybir.AxisListType.X,
                op=Op.add)
            nc.sync.dma_start(outp[:], out_t[:])

    nc.compile()
    return nc


def _prepare_in_maps(X, t):
    perm = np.argsort(t, kind="stable")
    Xs = X[perm]
    ts = t[perm]
    counts = np.bincount(ts.astype(np.int64))
    maxc = int(counts.max()) if counts.size else 0
    assert maxc <= 64, f"class count {maxc} exceeds band half-width 64"
    XT = np.ascontiguousarray(Xs.T).astype(np.float16)  # [D, N]
    tsf = ts.astype(np.float16)                         # exact for ids < 2048

    in_maps = []
    for c in range(NCORES):
        r0 = c * RPC
        lhs = np.ascontiguousarray(XT[:, r0:r0 + RPC])
        bwin = np.empty((D, MT * BW), np.float16)
        wlab = np.empty((MT * 128, BW), np.float16)
        rlab = np.empty((128, MT), np.float32)
        for j in range(MT):
            p = r0 + j * 128
            w0 = min(max(p - 64, 0), N - BW)
            bwin[:, j * BW:(j + 1) * BW] = XT[:, w0:w0 + BW]
            wlab[j * 128:(j + 1) * 128, :] = tsf[w0:w0 + BW][None, :]
            rlab[:, j] = tsf[p:p + 128]
        in_maps.append({
            "xt": XT, "lhs": lhs, "bwin": bwin, "wlab": wlab, "rlab": rlab,
        })
    return in_maps


def _reduce_outputs(results):
    tot_r = 0.0
    tot_s = 0.0
    tot_b = 0.0
    for c in range(NCORES):
        o = np.asarray(results[c]["out"], np.float64)
        tot_r += o[:, 0].sum()
        tot_s += o[:, 1].sum()
        tot_b += o[:, 2].sum()
    count_gt = (tot_s + float(N) * float(N)) / 2.0
    loss = (tot_r + MARGIN * count_gt + tot_b) / float(N)
    return np.float32(loss)


def kernel(inputs, targets, _trace=False, _tmpdir=None):
    X = np.asarray(inputs, dtype=np.float32)
    t = np.asarray(targets)
    assert X.shape == (N, D)

    if "nc" not in _CACHE:
        _CACHE["nc"] = _build_program()
    nc = _CACHE["nc"]

    in_maps = _prepare_in_maps(X, t)
    res = run_bass_kernel_spmd(
        nc, in_maps, list(range(NCORES)), trace=_trace, tmpdir=_tmpdir)
    loss = _reduce_outputs(res.results)
    if _trace:
        return loss, res
    return loss


# revision 7
# speedup vs baseline: 1.1874x; 1.1874x over previous
"""Trainium2 Bass kernel for nn_ContrastiveLoss (N=8192, D=1024, 751 ids).

loss = (1/N) * sum_ij [ same(i,j) & sim<1 -> (1-sim) ; diff(i,j) & sim>0.3 -> sim ]
with sim = X @ X.T.

Strategy (8 NeuronCores):
  * Host: sort rows by label (loss is permutation invariant). Same-label
    pairs then live within +-63 of the diagonal (max class count ~28).
  * sim is symmetric -> only the upper block-triangle is computed:
    16 row-blocks of 512 -> 136 block-pairs (a<=b), exactly 17 per core
    (core c takes block-rows c and 15-c). Off-diagonal pairs weigh 2x.
  * Per block-pair: fp16 matmul (fp32 PSUM). Unmasked sums need no label
    mask:  sum_j sim*1[sim>0.3] = sum relu(sim-0.3) + 0.3*count(sim>0.3).
    relu+accumulate on ScalarE (fused accum_out), count on VectorE
    (tensor_scalar is_gt with fused accum_out).
  * Band correction (256-wide windows around the diagonal): for
    same-label pairs subtract the neg term and add relu(1-sim), with a
    device-side label-equality mask.
  * Host: gather per-item partial sums, weight (1x diag / 2x off-diag),
    reduce in float64.
"""

import sys

for _p in ("/opt/trn_rl_repo",):
    if _p not in sys.path:
        sys.path.append(_p)

import numpy as np

import concourse.bass as bass
import concourse.mybir as mybir
import concourse.tile as tile
from concourse import bacc
from concourse.bass_utils import run_bass_kernel_spmd

N = 8192           # rows
D = 1024           # feature dim
NCORES = 8
B = 512            # triangle block size
NB = N // B        # 16 block-rows
NIT = 17           # items (block-pairs) per core
MS = B // 128      # m-subtiles per item = 4
KT = D // 128      # contraction chunks = 8
MT = (N // NCORES) // 128  # band row-tiles per core = 8
BW = 256           # band window width
MARGIN = 0.3

f16 = mybir.dt.float16
f32 = mybir.dt.float32

# output columns: per-item relu sums [0,68), per-item counts [68,136),
# band corr [136,144)
C_R = 0
C_C = NIT * MS          # 68
C_B = 2 * NIT * MS      # 136
C_OUT = C_B + MT        # 144

_CACHE = {}


def _core_items(c):
    """Block-pair list for core c: rows c and 15-c of the triangle."""
    items = [(c, b) for b in range(c, NB)]
    items += [(NB - 1 - c, b) for b in range(NB - 1 - c, NB)]
    assert len(items) == NIT
    return items


def _build_program():
    nc = bacc.Bacc("TRN2", target_bir_lowering=False, debug=False,
                   num_devices=NCORES)

    lhsd = nc.dram_tensor("lhsp", [D, NIT * B], f16, kind="ExternalInput")
    rhsd = nc.dram_tensor("rhsp", [D, NIT * B], f16, kind="ExternalInput")
    blhs = nc.dram_tensor("blhs", [D, MT * 128], f16, kind="ExternalInput")
    bwin = nc.dram_tensor("bwin", [D, MT * BW], f16, kind="ExternalInput")
    wlab = nc.dram_tensor("wlab", [MT * 128, BW], f16, kind="ExternalInput")
    rlab = nc.dram_tensor("rlab", [128, MT], f32, kind="ExternalInput")
    outp = nc.dram_tensor("out", [128, C_OUT], f32, kind="ExternalOutput")

    lhs_t = lhsd.rearrange("(k p) m -> k p m", p=128)
    rhs_t = rhsd.rearrange("(k p) n -> k p n", p=128)
    blhs_t = blhs.rearrange("(k p) m -> k p m", p=128)
    bwin_t = bwin.rearrange("(k p) w -> k p w", p=128)
    wlab_t = wlab.rearrange("(j p) w -> j p w", p=128)

    Relu = mybir.ActivationFunctionType.Relu
    Op = mybir.AluOpType

    with tile.TileContext(nc) as tc:
        with (
            tc.tile_pool(name="persist", bufs=1) as persist,
            tc.tile_pool(name="lhspool", bufs=2) as lhspool,
            tc.tile_pool(name="rhspool", bufs=2) as rhspool,
            tc.tile_pool(name="scr", bufs=8) as scr,
            tc.tile_pool(name="band", bufs=3) as bandp,
            tc.tile_pool(name="psum_m", bufs=6, space="PSUM") as psum_m,
            tc.tile_pool(name="psum_b", bufs=2, space="PSUM") as psum_b,
        ):
            # ---- persistent loads ----
            blhs_sb = []
            bwin_sb = []
            for k in range(KT):
                tb = persist.tile([128, MT * BW], f16, name=f"bwin{k}")
                nc.sync.dma_start(tb[:], bwin_t[k])
                bwin_sb.append(tb)
                tl = persist.tile([128, MT * 128], f16, name=f"blhs{k}")
                nc.sync.dma_start(tl[:], blhs_t[k])
                blhs_sb.append(tl)
            wlab_sb = []
            for j in range(MT):
                tw = persist.tile([128, BW], f16, name=f"wlab{j}")
                nc.sync.dma_start(tw[:], wlab_t[j])
                wlab_sb.append(tw)
            rlab_sb = persist.tile([128, MT], f32, name="rlab")
            nc.sync.dma_start(rlab_sb[:], rlab[:])

            stats = persist.tile([128, C_OUT], f32, name="stats")
            bias_m = persist.tile([128, 1], f32, name="bias_m")
            nc.vector.memset(bias_m[:], -MARGIN)

            # ---- band: sim on [128 x 256] diagonal windows ----
            for j in range(MT):
                ps = psum_b.tile([128, BW], f32)
                for k in range(KT):
                    nc.tensor.matmul(
                        ps[:],
                        blhs_sb[k][:, j * 128:(j + 1) * 128],
                        bwin_sb[k][:, j * BW:(j + 1) * BW],
                        start=(k == 0), stop=(k == KT - 1),
                    )
                pos = bandp.tile([128, BW], f32, name="pos")
                rb = bandp.tile([128, BW], f32, name="rb")
                gt = bandp.tile([128, BW], f32, name="gt")
                # pos = relu(1 - s);  rb = relu(s - 0.3);  gt = 1[s > 0.3]
                nc.scalar.activation(pos[:], ps[:], Relu, bias=1.0, scale=-1.0)
                nc.scalar.activation(rb[:], ps[:], Relu, bias=bias_m[:])
                nc.vector.tensor_scalar(gt[:], ps[:], MARGIN, None,
                                        op0=Op.is_gt)
                # neg = rb + 0.3*gt ; corr = eq * (pos - neg)
                a = bandp.tile([128, BW], f32, name="a")
                nc.vector.scalar_tensor_tensor(
                    a[:], gt[:], MARGIN, pos[:], op0=Op.mult, op1=Op.subtract)
                b = bandp.tile([128, BW], f32, name="b")
                nc.vector.tensor_tensor(b[:], a[:], rb[:], op=Op.add)
                # b = neg - pos
                eq = bandp.tile([128, BW], f32, name="eq")
                nc.vector.tensor_scalar(
                    eq[:], wlab_sb[j][:], rlab_sb[:, j:j + 1], None,
                    op0=Op.is_equal)
                crr = bandp.tile([128, BW], f32, name="crr")
                nc.vector.scalar_tensor_tensor(
                    crr[:], b[:], -1.0, eq[:], op0=Op.mult, op1=Op.mult,
                    accum_out=stats[:, C_B + j:C_B + j + 1])

            # ---- triangle sweep: 17 block-pairs of [512 x 512] ----
            for it in range(NIT):
                lhs_it = []
                rhs_it = []
                for k in range(KT):
                    tl = lhspool.tile([128, B], f16, name=f"lhs{k}")
                    nc.sync.dma_start(
                        tl[:], lhs_t[k, :, it * B:(it + 1) * B])
                    lhs_it.append(tl)
                    tr = rhspool.tile([128, B], f16, name=f"rhs{k}")
                    nc.sync.dma_start(
                        tr[:], rhs_t[k, :, it * B:(it + 1) * B])
                    rhs_it.append(tr)
                for m in range(MS):
                    ps = psum_m.tile([128, B], f32, name="mm")
                    for k in range(KT):
                        nc.tensor.matmul(
                            ps[:],
                            lhs_it[k][:, m * 128:(m + 1) * 128],
                            rhs_it[k][:],
                            start=(k == 0), stop=(k == KT - 1),
                        )
                    col = it * MS + m
                    sr = scr.tile([128, B], f16, name="sr")
                    nc.scalar.activation(
                        sr[:], ps[:], Relu, bias=bias_m[:],
                        accum_out=stats[:, C_R + col:C_R + col + 1])
                    sc = scr.tile([128, B], f16, name="sc")
                    nc.vector.tensor_scalar(
                        sc[:], ps[:], MARGIN, None, op0=Op.is_gt, op1=Op.add,
                        accum_out=stats[:, C_C + col:C_C + col + 1])

            nc.sync.dma_start(outp[:], stats[:])

    nc.compile()
    return nc


def _prepare_in_maps(X, t):
    perm = np.argsort(t, kind="stable")
    Xs = X[perm]
    ts = t[perm]
    counts = np.bincount(ts.astype(np.int64))
    maxc = int(counts.max()) if counts.size else 0
    assert maxc <= 64, f"class count {maxc} exceeds band half-width 64"
    XT = np.ascontiguousarray(Xs.T).astype(np.float16)  # [D, N]
    tsf = ts.astype(np.float16)                         # exact for ids < 2048

    in_maps = []
    weights = []
    for c in range(NCORES):
        items = _core_items(c)
        lhsp = np.empty((D, NIT * B), np.float16)
        rhsp = np.empty((D, NIT * B), np.float16)
        w = np.empty(NIT, np.float64)
        for i, (a, b) in enumerate(items):
            lhsp[:, i * B:(i + 1) * B] = XT[:, a * B:(a + 1) * B]
            rhsp[:, i * B:(i + 1) * B] = XT[:, b * B:(b + 1) * B]
            w[i] = 1.0 if a == b else 2.0
        weights.append(w)

        r0 = c * (N // NCORES)
        blhs = np.ascontiguousarray(XT[:, r0:r0 + MT * 128])
        bwin = np.empty((D, MT * BW), np.float16)
        wlaba = np.empty((MT * 128, BW), np.float16)
        rlab = np.empty((128, MT), np.float32)
        for j in range(MT):
            p = r0 + j * 128
            w0 = min(max(p - 64, 0), N - BW)
            bwin[:, j * BW:(j + 1) * BW] = XT[:, w0:w0 + BW]
            wlaba[j * 128:(j + 1) * 128, :] = tsf[w0:w0 + BW][None, :]
            rlab[:, j] = ts[p:p + 128].astype(np.float32)
        in_maps.append({
            "lhsp": lhsp, "rhsp": rhsp, "blhs": blhs, "bwin": bwin,
            "wlab": wlaba, "rlab": rlab,
        })
    return in_maps, weights


def _reduce_outputs(results, weights):
    tot = 0.0
    for c in range(NCORES):
        o = np.asarray(results[c]["out"], np.float64)
        r_items = o[:, C_R:C_C].sum(axis=0).reshape(NIT, MS).sum(axis=1)
        c_items = o[:, C_C:C_B].sum(axis=0).reshape(NIT, MS).sum(axis=1)
        neg_items = r_items + MARGIN * c_items
        tot += float((weights[c] * neg_items).sum())
        tot += float(o[:, C_B:C_OUT].sum())
    return np.float32(tot / float(N))


def kernel(inputs, targets, _trace=False, _tmpdir=None):
    X = np.asarray(inputs, dtype=np.float32)
    t = np.asarray(targets)
    assert X.shape == (N, D)

    if "nc" not in _CACHE:
        _CACHE["nc"] = _build_program()
    nc = _CACHE["nc"]

    in_maps, weights = _prepare_in_maps(X, t)
    res = run_bass_kernel_spmd(
        nc, in_maps, list(range(NCORES)), trace=_trace, tmpdir=_tmpdir)
    loss = _reduce_outputs(res.results, weights)
    if _trace:
        return loss, res
    return loss


# revision 8
# speedup vs baseline: 1.6538x; 1.3928x over previous
"""Trainium2 Bass kernel for nn_ContrastiveLoss (N=8192, D=1024, 751 ids).

loss = (1/N) * sum_ij [ same(i,j) & sim<1 -> (1-sim) ; diff(i,j) & sim>0.3 -> sim ]
with sim = X @ X.T.

Strategy (8 NeuronCores):
  * Host: sort rows by label (loss is permutation invariant). Same-label
    pairs then live within +-63 of the diagonal (max class count ~28).
  * sim is symmetric -> only the upper block-triangle is computed:
    16 row-blocks of 512 -> 136 block-pairs (a<=b), exactly 17 per core
    (core c takes block-rows c and 15-c). Off-diagonal pairs weigh 2x.
  * Per block-pair: fp16 matmul (fp32 PSUM). Unmasked sums need no label
    mask:  sum_j sim*1[sim>0.3] = sum relu(sim-0.3) + 0.3*count(sim>0.3).
    relu+accumulate on ScalarE (fused accum_out), count on VectorE
    (tensor_scalar is_gt with fused accum_out).
  * Band correction (256-wide windows around the diagonal): for
    same-label pairs subtract the neg term and add relu(1-sim), with a
    device-side label-equality mask.
  * Host: gather per-item partial sums, weight (1x diag / 2x off-diag),
    reduce in float64.
"""

import sys

for _p in ("/opt/trn_rl_repo",):
    if _p not in sys.path:
        sys.path.append(_p)

import numpy as np

import concourse.bass as bass
import concourse.mybir as mybir
import concourse.tile as tile
from concourse import bacc
from concourse.bass_utils import run_bass_kernel_spmd

N = 8192           # rows
D = 1024           # feature dim
NCORES = 8
B = 512            # triangle block size
NB = N // B        # 16 block-rows
NIT = 17           # items (block-pairs) per core
MS = B // 128      # m-subtiles per item = 4
KT = D // 128      # contraction chunks = 8
MT = (N // NCORES) // 128  # band row-tiles per core = 8
BW = 256           # band window width
IW = 2 * B         # packed item width (lhs 512 | rhs 512)
MARGIN = 0.3

# item pair groups for wide DMA streaming
GROUPS = [(g, min(2, NIT - g)) for g in range(0, NIT, 2)]

f16 = mybir.dt.float16
f32 = mybir.dt.float32

# output columns: per-item relu sums [0,68), per-item counts [68,136),
# band corr [136,144)
C_R = 0
C_C = NIT * MS          # 68
C_B = 2 * NIT * MS      # 136
C_OUT = C_B + MT        # 144

_CACHE = {}


def _core_items(c):
    """Block-pair list for core c: rows c and 15-c of the triangle."""
    items = [(c, b) for b in range(c, NB)]
    items += [(NB - 1 - c, b) for b in range(NB - 1 - c, NB)]
    assert len(items) == NIT
    return items


def _build_program():
    nc = bacc.Bacc("TRN2", target_bir_lowering=False, debug=False,
                   num_devices=NCORES)

    itemd = nc.dram_tensor("items", [D, NIT * IW], f16, kind="ExternalInput")
    blhs = nc.dram_tensor("blhs", [D, MT * 128], f16, kind="ExternalInput")
    bwin = nc.dram_tensor("bwin", [D, MT * BW], f16, kind="ExternalInput")
    wlab = nc.dram_tensor("wlab", [128, MT * BW], f16, kind="ExternalInput")
    rlab = nc.dram_tensor("rlab", [128, MT], f32, kind="ExternalInput")
    outp = nc.dram_tensor("out", [128, C_OUT], f32, kind="ExternalOutput")

    item_t = itemd.rearrange("(k p) m -> k p m", p=128)
    blhs_t = blhs.rearrange("(k p) m -> k p m", p=128)
    bwin_t = bwin.rearrange("(k p) w -> k p w", p=128)

    Relu = mybir.ActivationFunctionType.Relu
    Op = mybir.AluOpType

    with tile.TileContext(nc) as tc:
        with (
            tc.tile_pool(name="persist", bufs=1) as persist,
            tc.tile_pool(name="grp", bufs=2) as grpp,
            tc.tile_pool(name="scr", bufs=8) as scr,
            tc.tile_pool(name="band", bufs=3) as bandp,
            tc.tile_pool(name="psum_m", bufs=6, space="PSUM") as psum_m,
            tc.tile_pool(name="psum_b", bufs=2, space="PSUM") as psum_b,
        ):
            # ---- persistent loads ----
            blhs_sb = []
            bwin_sb = []
            for k in range(KT):
                tb = persist.tile([128, MT * BW], f16, name=f"bwin{k}")
                nc.sync.dma_start(tb[:], bwin_t[k])
                bwin_sb.append(tb)
                tl = persist.tile([128, MT * 128], f16, name=f"blhs{k}")
                nc.sync.dma_start(tl[:], blhs_t[k])
                blhs_sb.append(tl)
            wlab_sb = persist.tile([128, MT * BW], f16, name="wlab")
            nc.sync.dma_start(wlab_sb[:], wlab[:])
            rlab_sb = persist.tile([128, MT], f32, name="rlab")
            nc.sync.dma_start(rlab_sb[:], rlab[:])

            stats = persist.tile([128, C_OUT], f32, name="stats")
            bias_m = persist.tile([128, 1], f32, name="bias_m")
            nc.vector.memset(bias_m[:], -MARGIN)

            # ---- band: sim on [128 x 256] diagonal windows ----
            for j in range(MT):
                ps = psum_b.tile([128, BW], f32)
                for k in range(KT):
                    nc.tensor.matmul(
                        ps[:],
                        blhs_sb[k][:, j * 128:(j + 1) * 128],
                        bwin_sb[k][:, j * BW:(j + 1) * BW],
                        start=(k == 0), stop=(k == KT - 1),
                    )
                pos = bandp.tile([128, BW], f32, name="pos")
                rb = bandp.tile([128, BW], f32, name="rb")
                gt = bandp.tile([128, BW], f32, name="gt")
                # pos = relu(1 - s);  rb = relu(s - 0.3);  gt = 1[s > 0.3]
                nc.scalar.activation(pos[:], ps[:], Relu, bias=1.0, scale=-1.0)
                nc.scalar.activation(rb[:], ps[:], Relu, bias=bias_m[:])
                nc.vector.tensor_scalar(gt[:], ps[:], MARGIN, None,
                                        op0=Op.is_gt)
                # neg = rb + 0.3*gt ; corr = eq * (pos - neg)
                a = bandp.tile([128, BW], f32, name="a")
                nc.vector.scalar_tensor_tensor(
                    a[:], gt[:], MARGIN, pos[:], op0=Op.mult, op1=Op.subtract)
                b = bandp.tile([128, BW], f32, name="b")
                nc.vector.tensor_tensor(b[:], a[:], rb[:], op=Op.add)
                # b = neg - pos
                eq = bandp.tile([128, BW], f32, name="eq")
                nc.vector.tensor_scalar(
                    eq[:], wlab_sb[:, j * BW:(j + 1) * BW],
                    rlab_sb[:, j:j + 1], None, op0=Op.is_equal)
                crr = bandp.tile([128, BW], f32, name="crr")
                nc.vector.scalar_tensor_tensor(
                    crr[:], b[:], -1.0, eq[:], op0=Op.mult, op1=Op.mult,
                    accum_out=stats[:, C_B + j:C_B + j + 1])

            # ---- triangle sweep: 17 block-pairs of [512 x 512] ----
            for g0, gw in GROUPS:
                gq = []
                for k in range(KT):
                    tg = grpp.tile([128, 2 * IW], f16, name=f"gq{k}")
                    nc.sync.dma_start(
                        tg[:, :gw * IW],
                        item_t[k, :, g0 * IW:(g0 + gw) * IW])
                    gq.append(tg)
                for ii in range(gw):
                    off = ii * IW
                    for m in range(MS):
                        ps = psum_m.tile([128, B], f32, name="mm")
                        for k in range(KT):
                            nc.tensor.matmul(
                                ps[:],
                                gq[k][:, off + m * 128:off + (m + 1) * 128],
                                gq[k][:, off + B:off + IW],
                                start=(k == 0), stop=(k == KT - 1),
                            )
                        col = (g0 + ii) * MS + m
                        sr = scr.tile([128, B], f16, name="sr")
                        nc.scalar.activation(
                            sr[:], ps[:], Relu, bias=bias_m[:],
                            accum_out=stats[:, C_R + col:C_R + col + 1])
                        sc = scr.tile([128, B], f16, name="sc")
                        nc.vector.tensor_scalar(
                            sc[:], ps[:], MARGIN, None, op0=Op.is_gt,
                            op1=Op.add,
                            accum_out=stats[:, C_C + col:C_C + col + 1])

            nc.sync.dma_start(outp[:], stats[:])

    nc.compile()
    return nc


def _prepare_in_maps(X, t):
    perm = np.argsort(t, kind="stable")
    Xs = X[perm]
    ts = t[perm]
    counts = np.bincount(ts.astype(np.int64))
    maxc = int(counts.max()) if counts.size else 0
    assert maxc <= 64, f"class count {maxc} exceeds band half-width 64"
    XT = np.ascontiguousarray(Xs.T).astype(np.float16)  # [D, N]
    tsf = ts.astype(np.float16)                         # exact for ids < 2048

    in_maps = []
    weights = []
    for c in range(NCORES):
        items = _core_items(c)
        itemp = np.empty((D, NIT * IW), np.float16)
        w = np.empty(NIT, np.float64)
        for i, (a, b) in enumerate(items):
            itemp[:, i * IW:i * IW + B] = XT[:, a * B:(a + 1) * B]
            itemp[:, i * IW + B:(i + 1) * IW] = XT[:, b * B:(b + 1) * B]
            w[i] = 1.0 if a == b else 2.0
        weights.append(w)

        r0 = c * (N // NCORES)
        blhs = np.ascontiguousarray(XT[:, r0:r0 + MT * 128])
        bwin = np.empty((D, MT * BW), np.float16)
        wlaba = np.empty((128, MT * BW), np.float16)
        rlab = np.empty((128, MT), np.float32)
        for j in range(MT):
            p = r0 + j * 128
            w0 = min(max(p - 64, 0), N - BW)
            bwin[:, j * BW:(j + 1) * BW] = XT[:, w0:w0 + BW]
            wlaba[:, j * BW:(j + 1) * BW] = tsf[w0:w0 + BW][None, :]
            rlab[:, j] = ts[p:p + 128].astype(np.float32)
        in_maps.append({
            "items": itemp, "blhs": blhs, "bwin": bwin,
            "wlab": wlaba, "rlab": rlab,
        })
    return in_maps, weights


def _reduce_outputs(results, weights):
    tot = 0.0
    for c in range(NCORES):
        o = np.asarray(results[c]["out"], np.float64)
        r_items = o[:, C_R:C_C].sum(axis=0).reshape(NIT, MS).sum(axis=1)
        c_items = o[:, C_C:C_B].sum(axis=0).reshape(NIT, MS).sum(axis=1)
        neg_items = r_items + MARGIN * c_items
        tot += float((weights[c] * neg_items).sum())
        tot += float(o[:, C_B:C_OUT].sum())
    return np.float32(tot / float(N))


def kernel(inputs, targets, _trace=False, _tmpdir=None):
    X = np.asarray(inputs, dtype=np.float32)
    t = np.asarray(targets)
    assert X.shape == (N, D)

    if "nc" not in _CACHE:
        _CACHE["nc"] = _build_program()
    nc = _CACHE["nc"]

    in_maps, weights = _prepare_in_maps(X, t)
    res = run_bass_kernel_spmd(
        nc, in_maps, list(range(NCORES)), trace=_trace, tmpdir=_tmpdir)
    loss = _reduce_outputs(res.results, weights)
    if _trace:
        return loss, res
    return loss
